# revision 72
# baseline (speedup 1.0000x reference)
"""Trainium2 Bass kernel for nn_Encoding (vq_codebook).

Math (per batch b):
    xf = x[b].reshape(C, N).T                      # (N tokens, C)
    sl2[n,k] = scale[k] * (|xf_n|^2 - 2 xf_n.c_k + |c_k|^2)
    w = softmax_k(sl2)                             # max-subtract skipped: sl2 in (-600, -0.18]
    out[b] = w.T @ xf - (sum_n w)[:,None] * codewords

Sharding: data-parallel over batch B=32 -> 4 batches per core on 8 cores.
x is shipped to the device as bf16 (host cast): halves HBM traffic and
keeps rel err ~2e-3 against the 2e-2 gate (validated in fp64 emulation).

Per-core dataflow (unit = 2048 tokens; 2 units/batch, 8 units/core):
  - x loaded in natural (c-partition, token-free) bf16 layout, 512 KiB DMAs,
    prefetched one unit ahead.
  - |x|^2 entirely on PE: DVE squares xn into fp16 and pair-sums the two
    128-channel halves (2x mode, one unit ahead); 4 ones-basis matmuls
    reduce over channels into a (4 group, 512 token) PSUM tile (fp32-exact
    accumulation); after evac, one rank-4 f32r matmul folds scale_k * |x|^2
    into psl2.  No cross-layout shuffle needed.
  - PE is_transpose matmuls (bf16 identity -> 1 cyc/row) build xT tiles in
    bf16 PSUM; ACT/DVE/Pool evacuate them to SBUF for mm2.
  - mm1: psl2 (128 = 4 groups x 32 codes, 512 tokens) accumulates
    A = -2*scale*cw (bf16) against streamed bf16 x, one 32-col group per
    512-token group.
  - One ACT exp over (128, 512) with per-partition fp32 bias scale_k*|c_k|^2
    writes e as bf16.
  - Softmax denominators: PE matmul (bf16 group-indicator) -> (4, 512);
    DVE reciprocal; PE matmul broadcasts reciprocals back to (128, 512);
    DVE multiply normalizes -> w (bf16).
  - PE transposes w into (token, code) tiles (bf16 PSUM); DVE 2x-evacuates;
    mm2 (w stationary, xT moving, both bf16) accumulates out (32, 258) per
    batch; wsum rides col 256 via a ones-column in xT.
  - Final: one DVE scalar_tensor_tensor: out = cw*(-wsum) + wx; DMA out.
  - Unit u's softmax chain (exp..mm2) is interleaved into unit u+1's
    emission so each cross-engine hop overlaps transpose/mm1 work.
"""

import numpy as np
from contextlib import ExitStack

import ml_dtypes
import concourse.bass as bass
import concourse.bacc as bacc
import concourse.mybir as mybir
import concourse.tile as tile
from concourse.bass_utils import run_bass_kernel_spmd

F32 = mybir.dt.float32
F32R = mybir.dt.float32r
BF16 = mybir.dt.bfloat16
FP16 = mybir.dt.float16
ALU = mybir.AluOpType
ACTF = mybir.ActivationFunctionType

N_CORES = 8
B, C, K = 32, 256, 32
HW = 64 * 64            # 4096 tokens per batch
BL = B // N_CORES       # batches per core
UNIT = 2048             # tokens per unit
UNITS = BL * HW // UNIT  # 8 units per core
NCHUNK = 16             # 128-token chunks per unit
XTW = 258               # xT cols per chunk: 256 data + ones + pad


def build_module(bl=BL, debug=False):
    nc = bacc.Bacc(None)
    units = bl * HW // UNIT
    if debug:
        dbg_xT = nc.dram_tensor("dbg_xT", (128, NCHUNK * XTW), BF16, kind="ExternalOutput")
        dbg_e = nc.dram_tensor("dbg_e", (128, 512), BF16, kind="ExternalOutput")
        dbg_wt = nc.dram_tensor("dbg_wt", (128, 512), BF16, kind="ExternalOutput")
        dbg_wtT = nc.dram_tensor("dbg_wtT", (128, 512), BF16, kind="ExternalOutput")

    x_d = nc.dram_tensor("x", (bl, 2, 128, HW), BF16, kind="ExternalInput")
    a_d = nc.dram_tensor("A", (128, 8, 128), BF16, kind="ExternalInput")
    onb_d = nc.dram_tensor("ONB", (128, 4, 128), FP16, kind="ExternalInput")
    scl_d = nc.dram_tensor("SCL", (128, 1), F32, kind="ExternalInput")
    bias_d = nc.dram_tensor("BIASB", (128, 1), F32, kind="ExternalInput")
    gs_d = nc.dram_tensor("GS", (128, 4), BF16, kind="ExternalInput")
    gb_d = nc.dram_tensor("GB", (4, 128), BF16, kind="ExternalInput")
    cw_d = nc.dram_tensor("CWD", (32, 256), F32, kind="ExternalInput")
    onz_d = nc.dram_tensor("ONZ", (128, 32), BF16, kind="ExternalInput")
    idt_d = nc.dram_tensor("IDT", (128, 128), BF16, kind="ExternalInput")
    out_d = nc.dram_tensor("out", (bl, 32, 256), F32, kind="ExternalOutput")

    with tile.TileContext(nc) as tc, ExitStack() as ctx:
        sb = ctx.enter_context(tc.tile_pool(name="sb", bufs=2))
        sbx = ctx.enter_context(tc.tile_pool(name="sbx", bufs=4))
        cp = ctx.enter_context(tc.tile_pool(name="consts", bufs=1))
        ps_xt = ctx.enter_context(tc.tile_pool(name="ps_xt", bufs=3, space="PSUM"))
        ps_big = ctx.enter_context(tc.tile_pool(name="ps_big", bufs=2, space="PSUM"))
        ps_d = ctx.enter_context(tc.tile_pool(name="ps_d", bufs=1, space="PSUM"))
        ps_wtt = ctx.enter_context(tc.tile_pool(name="ps_wtt", bufs=1, space="PSUM"))
        ps_wx = ctx.enter_context(tc.tile_pool(name="ps_wx", bufs=1, space="PSUM"))

        def c(shape, dram, tag, dt):
            t = cp.tile(shape, dt, tag=tag)
            nc.sync.dma_start(t[:], dram[:])
            return t

        # the two tiny consts the first transposes/evacs need, then the
        # unit-0 x loads (they gate the pipeline), then the other consts.
        idt_s = c([128, 128], idt_d, "idt", BF16)
        onz_s = c([128, 32], onz_d, "onz", BF16)
        xn0 = sbx.tile([128, 2, UNIT], BF16, tag="xn")
        nc.sync.dma_start(xn0[:, 0], x_d[0, 0, :, 0:UNIT])
        nc.sync.dma_start(xn0[:, 1], x_d[0, 1, :, 0:UNIT])

        scl_s = c([128, 1], scl_d, "scl", F32)
        bias_s = c([128, 1], bias_d, "bias", F32)
        gs_s = c([128, 4], gs_d, "gs", BF16)
        gb_s = c([4, 128], gb_d, "gb", BF16)
        cw_s = c([32, 256], cw_d, "cw", F32)
        onb_s = cp.tile([128, 4, 128], FP16, tag="onb")
        nc.sync.dma_start(onb_s[:], onb_d[:])
        a_s = cp.tile([128, 8, 128], BF16, tag="a")
        nc.sync.dma_start(a_s[:], a_d[:])

        pwx = {}
        pending_out = []

        def load_xn(u, xn=None, first=False):
            """Load x natural, fill xT's ones columns, and square xn for the
            PE |x|^2 reduction.  All run one unit ahead of stage(u)."""
            b_, uu = u // 2, u % 2
            t0 = uu * UNIT
            if xn is None:
                xn = sbx.tile([128, 2, UNIT], BF16, tag="xn")
                nc.sync.dma_start(xn[:, 0], x_d[b_, 0, :, t0:t0 + UNIT])
                nc.sync.dma_start(xn[:, 1], x_d[b_, 1, :, t0:t0 + UNIT])
            xT = sbx.tile([128, NCHUNK * XTW], BF16, tag="xT")
            xTv = xT[:].rearrange("p (j c) -> p j c", c=XTW)
            # col 256 = ones (mm2 col 256 accumulates wsum), col 257 = pad.
            nc.gpsimd.tensor_copy(
                xTv[:, :, 256:258],
                onz_s[:].rearrange("p (j c) -> p j c", c=2))
            # xqs[c, n] = xn0[c,n]^2 + xn1[c,n]^2 (fp16, DVE 2x): halves
            # the PE channel-reduction matmuls for |x|^2.  For the first
            # unit skip the pair-sum: 8 q-matmuls on the raw squares start
            # the pipeline ~3 us earlier (xq0 is ready right after the
            # first DMA half lands).
            xq0 = sb.tile([128, UNIT], FP16, tag="xq0")
            nc.vector.tensor_tensor(xq0[:], xn[:, 0], xn[:, 0], ALU.mult)
            xq1 = sb.tile([128, UNIT], FP16, tag="xq1")
            nc.vector.tensor_tensor(xq1[:], xn[:, 1], xn[:, 1], ALU.mult)
            if first:
                return xn, xT, (xq0, xq1)
            xq = sbx.tile([128, UNIT], FP16, tag="xq")
            nc.vector.tensor_tensor(xq[:], xq0[:], xq1[:], ALU.add)
            return xn, xT, xq

        def stage(u, prev, xn, xT, xq):
            """Emit A(u) interleaved with B(prev)."""
            b_, uu = u // 2, u % 2

            xTv = xT[:].rearrange("p (j c) -> p j c", c=XTW)
            st = dict(xT=xT, b=b_, uu=uu, u=u)
            psl2 = ps_big.tile([128, 512], F32, tag="big")
            st["psl2"] = psl2

            def mm1_part(i, start=False, stop=False):
                g, cc = divmod(i, 2)
                nc.tensor.matmul(
                    psl2[:, :],
                    a_s[:, cc * 4 + g, :],
                    xn[:, cc, g * 512:(g + 1) * 512],
                    start=start, stop=stop, skip_group_check=True,
                )

            def q_part(g, start=False, stop=False):
                # psl2[32g+k, n'] += sum_c xqs[c, 512g + n']  (exact fp32,
                # unscaled; exp applies scale_k per partition)
                if isinstance(xq, tuple):   # first unit: raw per-cc squares
                    for cc in (0, 1):
                        nc.tensor.matmul(
                            psl2[:, :],
                            onb_s[:, g, :],
                            xq[cc][:, g * 512:(g + 1) * 512],
                            start=(start and cc == 0),
                            stop=(stop and cc == 1), skip_group_check=True,
                        )
                else:
                    nc.tensor.matmul(
                        psl2[:, :],
                        onb_s[:, g, :],
                        xq[:, g * 512:(g + 1) * 512],
                        start=start, stop=stop, skip_group_check=True,
                    )

            def tgroup(j2):
                # PE transposes for both cc halves of 2 chunks
                xtp = ps_xt.tile([128, 512], BF16, tag="xt")
                for h in (0, 1):
                    j = 2 * j2 + h
                    for cc in (0, 1):
                        nc.tensor.transpose(
                            xtp[:, h * 256 + cc * 128:h * 256 + cc * 128 + 128],
                            xn[:, cc, j * 128:j * 128 + 128],
                            idt_s[:],
                        )
                # evacuate both chunks in one strided op
                dst = xTv[:, 2 * j2:2 * j2 + 2, 0:256]
                src = xtp[:].rearrange("p (h c) -> p h c", c=256)
                # unit 0: DVE is busy squaring x for the first |x|^2 pass,
                # so its evac tiles would stall the transpose rotation.
                if j2 in (0, 2, 3, 5, 6) or p is None:
                    nc.scalar.copy(dst, src)
                else:
                    nc.vector.tensor_copy(dst, src)

            p = prev  # may be None (first unit)

            # emit deferred batch-output stores: by now the STT that feeds
            # them has drained, so the DMA doesn't block the ACT sequencer.
            while pending_out:
                ob, outs = pending_out.pop(0)
                nc.scalar.dma_start(out_d[ob], outs[:])

            # |x|^2 channel-reduction matmuls accumulate straight into
            # psl2.  For unit 0 they are emitted late (xq(0) is still being
            # computed when PE starts; transposes/mm1 only need xn).
            if p is not None:
                for i in range(4):
                    q_part(i, start=(i == 0))
            tgroup(0)
            if p is not None:
                e = sb.tile([128, 512], BF16, tag="e")
                nc.scalar.activation(e[:], p["psl2"][:], ACTF.Exp,
                                     bias=bias_s[:], scale=scl_s[:])
            tgroup(1)
            mm1_part(0, start=(p is None))
            mm1_part(1)
            if p is not None:
                ps4 = ps_d.tile([4, 512], F32, tag="d")
                nc.tensor.matmul(ps4[:], gs_s[:], e[:])
            tgroup(2)
            mm1_part(2)
            if p is not None:
                r4 = sb.tile([4, 512], BF16, tag="r4")
                with nc.allow_low_precision(reason="1/d in bf16: per-token scale, cancels in out"):
                    nc.vector.reciprocal(r4[:], ps4[:])
            tgroup(3)
            mm1_part(3)
            if p is not None:
                pR = ps_big.tile([128, 512], F32, tag="big")
                nc.tensor.matmul(pR[:], gb_s[:], r4[:])
            tgroup(4)
            mm1_part(4)
            if p is not None:
                wt = sb.tile([128, 512], BF16, tag="wt")
                nc.vector.tensor_tensor(wt[:], e[:], pR[:], ALU.mult)
            tgroup(5)
            mm1_part(5)
            tgroup(6)
            mm1_part(6)
            if p is not None:
                if debug and p["u"] == 0:
                    nc.scalar.dma_start(dbg_xT[:], p["xT"][:])
                    nc.scalar.dma_start(dbg_e[:], e[:])
                    nc.scalar.dma_start(dbg_wt[:], wt[:])
                pwtT = ps_wtt.tile([128, 512], BF16, tag="wtt")
                for sl in range(4):
                    # transpose of the (128, 128) slice: column-block g of
                    # the result is wT for token-chunk j = 4*g + sl.
                    nc.tensor.transpose(
                        pwtT[:, 128 * sl:128 * sl + 128],
                        wt[:, 128 * sl:128 * sl + 128],
                        idt_s[:],
                    )
            if p is not None:
                wtTs = sb.tile([128, 512], BF16, tag="wtTs")
                nc.vector.tensor_copy(wtTs[:], pwtT[:])
                if debug and p["u"] == 0:
                    nc.scalar.dma_start(dbg_wtT[:], wtTs[:])
                emit_mm2(p, wtTs, 0, 12)
            tgroup(7)
            mm1_part(7, stop=(p is not None))
            if p is None:
                # unit 0: late q matmuls close the accumulation
                for i in range(4):
                    q_part(i, stop=(i == 3))
            if p is not None:
                emit_mm2(p, wtTs, 12, NCHUNK)
            # tail: prefetch + xbar + squares for u+1
            if u + 1 < units:
                nxt = load_xn(u + 1)
            else:
                nxt = (None, None, None)
            return st, nxt

        def emit_mm2(p, wtTs, j0=0, j1=NCHUNK):
            b_, uu, xT = p["b"], p["uu"], p["xT"]
            if uu == 0 and j0 == 0:
                pwx[b_] = ps_wx.tile([32, XTW], F32, tag="wx", name="pwx")
            for j in range(j0, j1):
                nc.tensor.matmul(
                    pwx[b_][:, 0:XTW],
                    wtTs[:, 128 * (j % 4) + 32 * (j // 4):128 * (j % 4) + 32 * (j // 4) + 32],
                    xT[:, XTW * j:XTW * j + XTW],
                    start=(uu == 0 and j == 0), stop=(uu == 1 and j == NCHUNK - 1),
                    skip_group_check=True,
                )
            if uu == 1 and j1 == NCHUNK:
                outs = sb.tile([32, 256], F32, tag="outs")
                nc.vector.scalar_tensor_tensor(
                    out=outs[:], in0=cw_s[:], scalar=pwx[b_][:, 256:257],
                    in1=pwx[b_][:, 0:256], op0=ALU.mult, op1=ALU.add,
                )
                pending_out.append((b_, outs))
                del pwx[b_]

        def stage_last(u, p, xn, xT, xq):
            """Final unit: close its logits early (q+mm1+fold front-loaded)
            and thread its softmax chain through the remaining transposes so
            the post-loop drain is short."""
            b_, uu = u // 2, u % 2
            xTv = xT[:].rearrange("p (j c) -> p j c", c=XTW)
            st = dict(xT=xT, b=b_, uu=uu, u=u)
            psl2 = ps_big.tile([128, 512], F32, tag="big")
            st["psl2"] = psl2

            def mm1_part(i, stop=False):
                g, cc = divmod(i, 2)
                nc.tensor.matmul(
                    psl2[:, :], a_s[:, cc * 4 + g, :],
                    xn[:, cc, g * 512:(g + 1) * 512],
                    start=False, stop=stop, skip_group_check=True,
                )

            def tgroup(j2):
                xtp = ps_xt.tile([128, 512], BF16, tag="xt")
                for h in (0, 1):
                    j = 2 * j2 + h
                    for cc in (0, 1):
                        nc.tensor.transpose(
                            xtp[:, h * 256 + cc * 128:h * 256 + cc * 128 + 128],
                            xn[:, cc, j * 128:j * 128 + 128], idt_s[:],
                        )
                dst = xTv[:, 2 * j2:2 * j2 + 2, 0:256]
                srcv = xtp[:].rearrange("p (h c) -> p h c", c=256)
                if j2 in (0, 2, 3, 5, 6):
                    nc.scalar.copy(dst, srcv)
                else:
                    nc.vector.tensor_copy(dst, srcv)

            while pending_out:
                ob, outs = pending_out.pop(0)
                nc.scalar.dma_start(out_d[ob], outs[:])

            e_p = sb.tile([128, 512], BF16, tag="e")
            nc.scalar.activation(e_p[:], p["psl2"][:], ACTF.Exp,
                                 bias=bias_s[:], scale=scl_s[:])
            for g in range(4):
                nc.tensor.matmul(
                    psl2[:, :], onb_s[:, g, :], xq[:, g * 512:(g + 1) * 512],
                    start=(g == 0), stop=False, skip_group_check=True,
                )
            for i in range(4):
                mm1_part(i)
            ps4_p = ps_d.tile([4, 512], F32, tag="d")
            nc.tensor.matmul(ps4_p[:], gs_s[:], e_p[:])
            for i in range(4, 8):
                mm1_part(i, stop=(i == 7))
            r4_p = sb.tile([4, 512], BF16, tag="r4")
            with nc.allow_low_precision(reason="1/d in bf16: per-token scale, cancels in out"):
                nc.vector.reciprocal(r4_p[:], ps4_p[:])
            e_u = sb.tile([128, 512], BF16, tag="e")
            nc.scalar.activation(e_u[:], psl2[:], ACTF.Exp,
                                 bias=bias_s[:], scale=scl_s[:])
            tgroup(0)
            tgroup(1)
            pR_p = ps_big.tile([128, 512], F32, tag="big")
            nc.tensor.matmul(pR_p[:], gb_s[:], r4_p[:])
            tgroup(2)
            wt_p = sb.tile([128, 512], BF16, tag="wt")
            nc.vector.tensor_tensor(wt_p[:], e_p[:], pR_p[:], ALU.mult)
            ps4_u = ps_d.tile([4, 512], F32, tag="d")
            nc.tensor.matmul(ps4_u[:], gs_s[:], e_u[:])
            tgroup(3)
            r4_u = sb.tile([4, 512], BF16, tag="r4")
            with nc.allow_low_precision(reason="1/d in bf16: per-token scale, cancels in out"):
                nc.vector.reciprocal(r4_u[:], ps4_u[:])
            pwtT_p = ps_wtt.tile([128, 512], BF16, tag="wtt")
            for sl in range(4):
                nc.tensor.transpose(
                    pwtT_p[:, 128 * sl:128 * sl + 128],
                    wt_p[:, 128 * sl:128 * sl + 128], idt_s[:],
                )
            tgroup(4)
            pR_u = ps_big.tile([128, 512], F32, tag="big")
            nc.tensor.matmul(pR_u[:], gb_s[:], r4_u[:])
            wtTs_p = sb.tile([128, 512], BF16, tag="wtTs")
            nc.vector.tensor_copy(wtTs_p[:], pwtT_p[:])
            tgroup(5)
            wt_u = sb.tile([128, 512], BF16, tag="wt")
            nc.vector.tensor_tensor(wt_u[:], e_u[:], pR_u[:], ALU.mult)
            emit_mm2(p, wtTs_p)
            tgroup(6)
            tgroup(7)
            return st, wt_u

        cur = load_xn(0, xn=xn0, first=True)
        prev = None
        for u in range(units - 1):
            prev, cur = stage(u, prev, *cur)
        prev, wt_last = stage_last(units - 1, prev, *cur)
        # epilogue: only the tail of the last unit's chain remains
        pwtT = ps_wtt.tile([128, 512], BF16, tag="wtt")
        for sl in range(4):
            nc.tensor.transpose(
                pwtT[:, 128 * sl:128 * sl + 128],
                wt_last[:, 128 * sl:128 * sl + 128],
                idt_s[:],
            )
        wtTs = sb.tile([128, 512], BF16, tag="wtTs")
        nc.vector.tensor_copy(wtTs[:], pwtT[:])
        emit_mm2(prev, wtTs)
        while pending_out:
            ob, outs = pending_out.pop(0)
            nc.scalar.dma_start(out_d[ob], outs[:])

    nc.finalize()
    return nc


def host_constants(codewords, scale):
    cw = np.asarray(codewords, dtype=np.float32)
    sc = np.asarray(scale, dtype=np.float32)
    c_sq = (cw.astype(np.float64) ** 2).sum(-1).astype(np.float32)

    A = np.zeros((2, 4, 128, 128), np.float32)
    for cc in range(2):
        blk = -2.0 * cw[:, cc * 128:(cc + 1) * 128].T
        for g in range(4):
            A[cc, g, :, 32 * g:32 * g + 32] = blk

    ONB = np.zeros((128, 4, 128), np.float32)
    SCL = np.zeros((128, 1), np.float32)
    BIASB = np.zeros((128, 1), np.float32)
    GS = np.zeros((128, 4), np.float32)
    GB = np.zeros((4, 128), np.float32)
    for g in range(4):
        ONB[:, g, 32 * g:32 * g + 32] = 1.0
        SCL[32 * g:32 * g + 32, 0] = sc
        BIASB[32 * g:32 * g + 32, 0] = sc * c_sq
        GS[32 * g:32 * g + 32, g] = 1.0
        GB[g, 32 * g:32 * g + 32] = 1.0

    bf = ml_dtypes.bfloat16
    return {
        "A": np.ascontiguousarray(A.transpose(2, 0, 1, 3).reshape(128, 8, 128)).astype(bf),
        "ONB": ONB.astype(np.float16), "SCL": SCL,
        "BIASB": BIASB, "GS": GS.astype(bf), "GB": GB.astype(bf),
        "CWD": np.ascontiguousarray(-cw),
        "ONZ": np.tile(np.array([1.0, 0.0], bf), (128, 16)),
        "IDT": np.eye(128, dtype=bf),
    }


def make_in_maps(x, codewords, scale):
    consts = host_constants(codewords, scale)
    xb = np.asarray(x).astype(ml_dtypes.bfloat16)
    xs = xb.reshape(B, 2, 128, HW)
    in_maps = []
    for i in range(N_CORES):
        m = dict(consts)
        m["x"] = np.ascontiguousarray(xs[BL * i:BL * (i + 1)])
        in_maps.append(m)
    return in_maps


_CACHE = {}


def kernel(x, codewords, scale):
    if "nc" not in _CACHE:
        _CACHE["nc"] = build_module()
    nc = _CACHE["nc"]
    in_maps = make_in_maps(x, codewords, scale)
    res = run_bass_kernel_spmd(nc, in_maps, list(range(N_CORES)))
    out = np.concatenate([r["out"] for r in res.results], axis=0)
    return out.astype(np.float32)
